# revision 13
# baseline (speedup 1.0000x reference)
"""Distributed causal attention head on 8 TRN2 NeuronCores.

Problem: B=4, S=4096, D_in=512, D_out=64 causal attention
  K/V/Q = X @ W; scores = Q@K^T (causal, /sqrt(64)); Z = softmax(scores)@V

Sharding: core c = 2*b + h handles batch b, seq-half h.
q-rows are interleaved at 128-row-block granularity (core h owns global
q-blocks {2j+h}), which makes the causal block schedule IDENTICAL on all
cores (SPMD-safe) and balances FLOPs exactly.  Every core loads the full
(transposed) K/V inputs of its batch and projects them locally.

Perf structure:
 - All host tensors are partition-major so every DMA descriptor is a
   1-8KB contiguous row (descriptor-processing, not bandwidth, limits
   badly-shaped DMAs).
 - Input DMAs are ordered by first-use time and split so each chunk's
   slices arrive just ahead of the compute that needs them; triggers are
   spread over the sync/scalar/gpsimd queues (a trigger occupies its
   queue ~0.6us and a queue's transfers serialize).
 - PE warmup spin (dummy matmuls on the weight tile) so the HAM clock
   gate is at 8/8 (2.4 GHz) when real projections start.
 - Wq/Wk are host-duplicated to [D, 128] so projections emit [128, 512]
   PSUM (both parity copies in one matmul + one CAST), feeding the
   row-tiled score matmuls directly.
 - Scores are computed transposed ST[k,q] with K=64 PAIRS row-tiled in
   the PE; exp on ACT in groups of 2 kblocks (scale=1/8 folded, no
   max-subtraction: |scores/8| < ~1.5); AV matmuls accumulate Z^T in
   PSUM with a ones-column in Vp giving the softmax denominator free;
   Z^T is PE-transposed back to q-major, normalized with a reciprocal +
   tensor_scalar_mul into a persistent [128, 16*64] output tile that is
   DMA'd per chunk (contiguous, host un-permutes).
"""

import numpy as np
import ml_dtypes

import concourse.bass as bass
import concourse.bacc as bacc
import concourse.mybir as mybir
import concourse.tile as tile

B, S, D, E = 4, 4096, 512, 64
PB = 128                      # partition block
NKB = S // PB                 # 32 k-blocks (global)
NLQ = NKB // 2                # 16 local q-blocks per core
NCH = 4                       # q-chunks of 512 per core
CHW = 512                     # q-chunk width
ND = D // PB                  # 4 d-slices
GRP = 2                       # kblocks per exp group
LAG = 4                       # ST->AV software pipeline depth (groups)
NWARM = 7                     # PE warmup matmuls (N=1024 each)
BF16 = mybir.dt.bfloat16
F8 = mybir.dt.float8e4
F32 = mybir.dt.float32
NPBF16 = ml_dtypes.bfloat16
NPF8 = ml_dtypes.float8_e4m3fn
WSCALE = 16.0


def build_nc():
    nc = bacc.Bacc(None)

    xq_d = nc.declare_dram_parameter("xq", [PB, 8192], F8, isOutput=False)
    xk_d = nc.declare_dram_parameter("xk", [PB, 16384], F8, isOutput=False)
    xv_d = nc.declare_dram_parameter("xv", [PB, 16384], BF16, isOutput=False)
    wq_d = nc.declare_dram_parameter("wq", [PB, ND * PB], F8, isOutput=False)
    wk_d = nc.declare_dram_parameter("wk", [PB, ND * PB], F8, isOutput=False)
    wv_d = nc.declare_dram_parameter("wv", [PB, ND * E], BF16, isOutput=False)
    cm_d = nc.declare_dram_parameter("cmask", [PB, 8 * CHW], BF16, isOutput=False)
    id_d = nc.declare_dram_parameter("ident", [PB, PB], F32, isOutput=False)
    out_d = nc.declare_dram_parameter("out", [PB, NLQ * E], F32, isOutput=True)

    with tile.TileContext(nc) as tc:
        with tc.tile_pool(name="persist", bufs=1) as pp, \
             tc.tile_pool(name="st_ps", bufs=2, space="PSUM") as stp, \
             tc.tile_pool(name="pj_ps", bufs=2, space="PSUM") as pjp, \
             tc.tile_pool(name="zt_ps", bufs=2, space="PSUM") as ztp, \
             tc.tile_pool(name="work", bufs=2 * LAG + 2) as wp:
            # ---- persistent SBUF tiles ----
            wq_sb = pp.tile([PB, ND * PB], F8, name="wq_sb", tag="wq_sb")
            wk_sb = pp.tile([PB, ND * PB], F8, name="wk_sb", tag="wk_sb")
            wv_sb = pp.tile([PB, ND * E], BF16, name="wv_sb", tag="wv_sb")
            mk_sb = pp.tile([PB, 8 * CHW], BF16, name="mk_sb", tag="mk_sb")
            idf_sb = pp.tile([PB, PB], F32, name="idf_sb", tag="idf_sb")
            idb_sb = pp.tile([PB, PB], BF16, name="idb_sb", tag="idb_sb")
            # inputs, split by first-use time (A: chunk 0, B1: chunk 1,
            # B2: chunks 2-3 / k-chunks 4-7); each tile holds all 4
            # d-slices side by side so one DMA covers it.
            xqA = pp.tile([PB, ND * CHW], F8, name="xqA", tag="xqA")
            xqB1 = pp.tile([PB, ND * CHW], F8, name="xqB1", tag="xqB1")
            xqB2 = pp.tile([PB, ND * 2 * CHW], F8, name="xqB2", tag="xqB2")
            xkA = pp.tile([PB, ND * 2 * CHW], F8, name="xkA", tag="xkA")
            xkB1 = pp.tile([PB, ND * 2 * CHW], F8, name="xkB1", tag="xkB1")
            xkB2 = pp.tile([PB, ND * 4 * CHW], F8, name="xkB2", tag="xkB2")
            xvA = pp.tile([PB, ND * 2 * CHW], BF16, name="xvA", tag="xvA")
            xvB1 = pp.tile([PB, ND * 2 * CHW], BF16, name="xvB1", tag="xvB1")
            xvB2 = pp.tile([PB, ND * 4 * CHW], BF16, name="xvB2", tag="xvB2")
            # projected tensors, chunked  (rows 0:64 == rows 64:128)
            qpT = [pp.tile([PB, CHW], BF16, name=f"qpT{c}", tag=f"qpT{c}")
                   for c in range(NCH)]
            kpT = [pp.tile([PB, CHW], BF16, name=f"kpT{c}", tag=f"kpT{c}")
                   for c in range(2 * NCH)]
            vpT = [pp.tile([E, CHW], BF16, name=f"vpT{c}", tag=f"vpT{c}")
                   for c in range(2 * NCH)]
            vp = [pp.tile([PB, E + 1], BF16, name=f"vp{s}", tag=f"vp{s}")
                  for s in range(NKB)]
            out_sb = pp.tile([PB, NLQ * E], F32, name="out_sb", tag="out_sb")

            def xq_ap(d, c):
                if c == 0:
                    return xqA[:, CHW * d:CHW * (d + 1)]
                if c == 1:
                    return xqB1[:, CHW * d:CHW * (d + 1)]
                w = 2 * CHW
                return xqB2[:, w * d + CHW * (c - 2):w * d + CHW * (c - 1)]

            def xk_ap(d, kc):
                w = 2 * CHW
                if kc < 2:
                    return xkA[:, w * d + CHW * kc:w * d + CHW * (kc + 1)]
                if kc < 4:
                    return xkB1[:, w * d + CHW * (kc - 2):w * d + CHW * (kc - 1)]
                w = 4 * CHW
                return xkB2[:, w * d + CHW * (kc - 4):w * d + CHW * (kc - 3)]

            def xv_ap(d, kc):
                w = 2 * CHW
                if kc < 2:
                    return xvA[:, w * d + CHW * kc:w * d + CHW * (kc + 1)]
                if kc < 4:
                    return xvB1[:, w * d + CHW * (kc - 2):w * d + CHW * (kc - 1)]
                w = 4 * CHW
                return xvB2[:, w * d + CHW * (kc - 4):w * d + CHW * (kc - 3)]

            # ---- input DMAs: by first-use time, explicit queues ----
            # host pre-merges d-slices, so each DMA is one contiguous
            # column range (128 descriptors of 2-16KB)
            dma_scalar = [
                (wq_sb[:], wq_d[:]),
                (xqA[:], xq_d[:, 0:2048]),
                (idf_sb[:], id_d[:]),
                (wv_sb[:], wv_d[:]),
                (mk_sb[:, 0:4 * CHW], cm_d[:, 0:4 * CHW]),
                (mk_sb[:, 4 * CHW:8 * CHW], cm_d[:, 4 * CHW:8 * CHW]),
            ]
            dma_sync = [
                (wk_sb[:], wk_d[:]),
                (xkA[:], xk_d[:, 0:4096]),
                (xqB1[:], xq_d[:, 2048:4096]),
                (xkB1[:], xk_d[:, 4096:8192]),
                (xqB2[:], xq_d[:, 4096:8192]),
                (xkB2[:], xk_d[:, 8192:16384]),
            ]
            dma_pool = [
                (xvA[:], xv_d[:, 0:4096]),
                (xvB1[:], xv_d[:, 4096:8192]),
                (xvB2[:], xv_d[:, 8192:16384]),
            ]
            for o, inp in dma_scalar:
                nc.scalar.dma_start(out=o, in_=inp)
            for o, inp in dma_sync:
                nc.sync.dma_start(out=o, in_=inp)
            for o, inp in dma_pool:
                nc.gpsimd.dma_start(out=o, in_=inp)

            warm_sb = pp.tile([PB, 2 * CHW], BF16, name="warm_sb", tag="warm_sb")
            nc.vector.memset(warm_sb[:], 0.125)
            nc.vector.tensor_copy(idb_sb[:], idf_sb[:])
            for s in range(NKB):
                nc.vector.memset(vp[s][:], 1.0)   # ones column prefill

            # ---- PE warmup: keep HAM busy until real projections start;
            # full-width writes also initialize the st PSUM banks ----
            for i in range(NWARM):
                wm_ps = stp.tile([PB, GRP * CHW], F32, tag="st")
                nc.tensor.matmul(wm_ps[:, 0:CHW], warm_sb[:, 0:PB], warm_sb[:, 0:CHW],
                                 start=True, stop=True)
                nc.tensor.matmul(wm_ps[:, CHW:2 * CHW], warm_sb[:, 0:PB], warm_sb[:, CHW:2 * CHW],
                                 start=True, stop=True)

            def vtrans(s):
                """PE-transpose one projected-V block to k-major + copy out."""
                vproj(s // 4)
                vt_ps = pjp.tile([PB, E], BF16, tag="pj")
                nc.tensor.transpose(vt_ps[:], vpT[s // 4][:, PB * (s % 4):PB * (s % 4 + 1)],
                                    idb_sb[0:E, 0:E])
                nc.vector.tensor_copy(vp[s][:, 0:E], vt_ps[:])

            def project(c):
                """Project Q chunk c and K chunks 2c, 2c+1 (V projections
                are emitted later, interleaved between ST groups)."""
                qp_ps = pjp.tile([PB, CHW], F32, tag="pj")
                for d in range(ND):
                    nc.tensor.matmul(qp_ps[:], wq_sb[:, PB * d:PB * (d + 1)],
                                     xq_ap(d, c),
                                     start=(d == 0), stop=(d == ND - 1))
                nc.vector.tensor_copy(qpT[c][:], qp_ps[:])
                for kc in (2 * c, 2 * c + 1):
                    kp_ps = pjp.tile([PB, CHW], F32, tag="pj")
                    for d in range(ND):
                        nc.tensor.matmul(kp_ps[:], wk_sb[:, PB * d:PB * (d + 1)],
                                         xk_ap(d, kc),
                                         start=(d == 0), stop=(d == ND - 1))
                    nc.vector.tensor_copy(kpT[kc][:], kp_ps[:])

            vproj_done = set()

            def vproj(kc):
                """Lazily project V chunk kc (called at first vtrans use)."""
                if kc in vproj_done:
                    return
                vproj_done.add(kc)
                vq_ps = pjp.tile([E, CHW], F32, tag="pj")
                for d in range(ND):
                    nc.tensor.matmul(vq_ps[:], wv_sb[:, E * d:E * (d + 1)],
                                     xv_ap(d, kc),
                                     start=(d == 0), stop=(d == ND - 1))
                nc.vector.tensor_copy(vpT[kc][:], vq_ps[:])

            QOFF = [0, 0, 128, 128, 256, 256, 384, 384]

            def qoff(kb, c):
                m = kb - 8 * c
                return QOFF[m] if m >= 0 else 0

            def st_mm(st_ps, ji, kb, c):
                pb = E * (kb % 2)
                kc, col = kb // 4, PB * (kb % 4)
                off = qoff(kb, c)
                nc.tensor.matmul(st_ps[:, CHW * ji + off:CHW * (ji + 1)],
                                 kpT[kc][pb:pb + E, col:col + PB],
                                 qpT[c][pb:pb + E, off:CHW],
                                 start=True, stop=True, tile_position=(pb, 0))

            # prologue: first chunk's projections
            project(0)

            for c in range(NCH):
                nkb = 8 * c + 8
                zt_ps = ztp.tile([E + 1, CHW], F32, tag="zt")
                korder = list(range(0, nkb))
                groups = [korder[i:i + GRP] for i in range(0, nkb, GRP)]
                pend = []
                drain_state = {"n": 0}

                def drain_avs(p_et, p_kbs, nkb=nkb, zt_ps=zt_ps, c=c, ds=drain_state):
                    for kb in p_kbs:      # late vtrans, spread across groups
                        if kb >= 8 * c:
                            vtrans(kb)
                    for ji, kb in enumerate(p_kbs):
                        off = qoff(kb, c)
                        nc.tensor.matmul(
                            zt_ps[:, off:CHW], vp[kb][:],
                            p_et[:, CHW * ji + off:CHW * (ji + 1)],
                            start=(ds["n"] == 0),
                            stop=(ds["n"] == nkb - 1),
                            skip_group_check=True)
                        ds["n"] += 1

                for gi, kbs in enumerate(groups):
                    gw = len(kbs) * CHW
                    st_ps = stp.tile([PB, GRP * CHW], F32, tag="st")
                    # pair of consecutive kblocks -> concurrent row-tiled MMs
                    if len(kbs) >= 2:
                        st_mm(st_ps, 0, kbs[0], c)
                        st_mm(st_ps, 1, kbs[1], c)
                        rest = range(2, len(kbs))
                    else:
                        rest = range(len(kbs))
                    for ji in rest:
                        st_mm(st_ps, ji, kbs[ji], c)
                    if len(pend) > LAG - 1:
                        drain_avs(*pend.pop(0))
                    et_sb = wp.tile([PB, GRP * CHW], BF16, tag="et")
                    off0 = qoff(kbs[0], c)
                    nc.scalar.activation(
                        et_sb[:, off0:gw], st_ps[:, off0:gw],
                        mybir.ActivationFunctionType.Exp, scale=0.125 / (WSCALE * WSCALE))
                    m0 = kbs[0] - 8 * c
                    if len(kbs) == 2 and m0 >= 0:
                        # both kblocks in the diag region: adjacent masks,
                        # one trimmed multiply
                        nc.vector.tensor_mul(
                            et_sb[:, off0:gw], et_sb[:, off0:gw],
                            mk_sb[:, CHW * m0 + off0:CHW * (m0 + 2)])
                    else:
                        for ji, kb in enumerate(kbs):
                            m = kb - 8 * c
                            if m >= 0:
                                nc.vector.tensor_mul(
                                    et_sb[:, CHW * ji:CHW * (ji + 1)],
                                    et_sb[:, CHW * ji:CHW * (ji + 1)],
                                    mk_sb[:, CHW * m:CHW * (m + 1)])
                    pend.append((et_sb, kbs))
                for p in pend:
                    drain_avs(*p)
                zs_sb = wp.tile([E + 1, CHW], F32, tag="zs")
                nc.vector.tensor_copy(zs_sb[:], zt_ps[:])
                # project next chunk while exp/AV tail of this chunk drains
                if c + 1 < NCH:
                    project(c + 1)
                # normalize via transpose (denominator = col E)
                for j in range(4):
                    zn_ps = ztp.tile([PB, E + 1], F32, tag="zt")
                    nc.tensor.transpose(zn_ps[:], zs_sb[:, PB * j:PB * (j + 1)],
                                        idf_sb[0:E + 1, 0:E + 1])
                    rc_sb = wp.tile([PB, 1], F32, tag="rc")
                    nc.vector.reciprocal(rc_sb[:], zn_ps[:, E:E + 1])
                    jj = 4 * c + j
                    nc.vector.tensor_scalar_mul(out_sb[:, E * jj:E * (jj + 1)],
                                                zn_ps[:, 0:E], rc_sb[:])
                # chunk's output block: one contiguous DMA of [128, 4*E] f32
                nc.gpsimd.dma_start(
                    out=out_d[:, 4 * E * c:4 * E * (c + 1)],
                    in_=out_sb[:, 4 * E * c:4 * E * (c + 1)])
    nc.finalize()
    return nc


def make_core_inputs(key_np, value_np, query_np, Wk, Wv, Wq):
    """Host-side sharding: returns in_maps list of 8 dicts."""
    bf = lambda a: np.ascontiguousarray(a).astype(NPBF16)

    def pmajor(w, width):
        """[D, width] -> [128, ND*width] partition-major."""
        return np.ascontiguousarray(
            w.reshape(ND, PB, width).transpose(1, 0, 2).reshape(PB, ND * width))

    f8 = lambda a: np.ascontiguousarray(a).astype(NPF8)
    Wqs, Wks = Wq * WSCALE, Wk * WSCALE
    wq2 = f8(pmajor(np.concatenate([Wqs, Wqs], axis=1), PB))
    wk2 = f8(pmajor(np.concatenate([Wks, Wks], axis=1), PB))
    wv2 = pmajor(Wv, E)
    in_maps = []
    for c in range(8):
        b, h = c // 2, c % 2
        qrows = np.concatenate(
            [np.arange(PB * (2 * j + h), PB * (2 * j + h) + PB) for j in range(NLQ)])
        # causal masks: mask m applies to kblock kb = 8c+m of every chunk;
        # section jj (q sub-block) has global q-block g = 8c+2jj+h,
        # class = m - 2jj - h: <0 keep, ==0 triangular, >0 zero.
        cmask = np.zeros((8, PB, CHW), dtype=np.float32)
        ki = np.arange(PB)[:, None]
        qi = np.arange(PB)[None, :]
        tri = (ki <= qi).astype(np.float32)
        for m in range(8):
            for jj in range(4):
                cls = m - 2 * jj - h
                blk = np.ones((PB, PB), np.float32) if cls < 0 else (
                    tri if cls == 0 else np.zeros((PB, PB), np.float32))
                cmask[m][:, PB * jj:PB * (jj + 1)] = blk
        cmask_pm = np.ascontiguousarray(
            cmask.transpose(1, 0, 2).reshape(PB, 8 * CHW))
        def merge_tiles(x, tiles):
            # x: [D, cols] -> [128, sum(ND*w)]: per tile, d-slices side
            # by side, partition-major (one contiguous row per partition)
            parts = []
            for lo, hi in tiles:
                w = hi - lo
                parts.append(x[:, lo:hi].reshape(ND, PB, w)
                             .transpose(1, 0, 2).reshape(PB, ND * w))
            return np.ascontiguousarray(np.concatenate(parts, axis=1))

        xq_m = merge_tiles(query_np[b][qrows].T,
                           [(0, CHW), (CHW, 2 * CHW), (2 * CHW, 4 * CHW)])
        xk_m = merge_tiles(key_np[b].T,
                           [(0, 2 * CHW), (2 * CHW, 4 * CHW), (4 * CHW, 8 * CHW)])
        xv_m = merge_tiles(value_np[b].T,
                           [(0, 2 * CHW), (2 * CHW, 4 * CHW), (4 * CHW, 8 * CHW)])
        in_maps.append({
            "xq": f8(xq_m),
            "xk": f8(xk_m),
            "xv": bf(xv_m),
            "wq": wq2, "wk": wk2, "wv": bf(wv2),
            "cmask": bf(cmask_pm),
            "ident": np.eye(PB, dtype=np.float32),
        })
    return in_maps


def assemble_output(results):
    """results: list of 8 dicts with 'out' [128, 16*64] f32 -> Z [B,S,E]."""
    Z = np.zeros((B, S, E), dtype=np.float32)
    for c in range(8):
        b, h = c // 2, c % 2
        o = results[c]["out"].reshape(PB, NLQ, E)  # [p, j, e]
        for j in range(NLQ):
            g = 2 * j + h
            Z[b, PB * g:PB * (g + 1), :] = o[:, j, :]
    return Z


def kernel(key_inputs, value_inputs, query_inputs, Wk, Wv, Wq):
    from concourse.bass_utils import run_bass_kernel_spmd
    nc = build_nc()
    in_maps = make_core_inputs(np.asarray(key_inputs), np.asarray(value_inputs),
                               np.asarray(query_inputs), np.asarray(Wk),
                               np.asarray(Wv), np.asarray(Wq))
    res = run_bass_kernel_spmd(nc, in_maps, core_ids=list(range(8)))
    return assemble_output(res.results)
